# revision 1
# baseline (speedup 1.0000x reference)
"""GNN message-passing layer (EGNN-style GCL) on 8 Trainium2 NeuronCores.

Strategy (per spec sharding hint): shard edges across the 8 cores BY
DESTINATION ROW (row = edge_index[0], the segment-sum target), so each core
owns a 2500-node output partition and its incoming edges. Per core:

  P0: z1a = h_my @ W_m1[0:128]         (node table, [2560,128] -> DRAM)
      z1b = h_all @ W_m1[128:256]      (node table, [20096,128] -> DRAM)
  P1 (edge loop, edges sorted by row, bucketed into 128-node windows):
      zrow = dma_gather(z1a, row_local)          # [128e,128d] tiles
      zcol = dma_gather(z1b, col)
      zea  = ea_aug @ W1c_aug                     # bias row folded in
      m1   = silu(zea + zrow + zcol)              # PSUM accumulate via
                                                  # identity-matmul pushes
      m2   = (m1^T via PE transpose) @ W_m2       # PSUM fp32
      att  = sigmoid(rowsum(m2 * W_a_bcast) + b_a)
      msg  = m2 * (att * edge_mask)               # bf16
      agg_fm[:, win] += msg^T @ onehot(row_local) # PE matmul scatter into
                                                  # per-window PSUM banks
  P2 (node MLP, feature-major):
      u1  = silu(W_n1h^T h_fm + W_n1a^T agg_fm + b_n1)
      upd = W_n2^T u1 + b_n2
      out = (h + upd^T) * flags

No collectives needed: every core's scatter targets only its own rows.
"""
import sys

for _p in ('/opt/trn_rl_repo', '/root/.axon_site/_ro/trn_rl_repo'):
    if _p not in sys.path:
        sys.path.append(_p)

import numpy as np
import ml_dtypes

from concourse import bacc
import concourse.mybir as mybir
import concourse.tile as tile

F32 = mybir.dt.float32
BF16 = mybir.dt.bfloat16
I16 = mybir.dt.int16

# problem constants (hardcoded per spec)
N_NODES, N_EDGES = 20000, 640000
D, ED = 128, 16
NCORES = 8

DEFAULT_CFG = dict(
    n_nodes=N_NODES,
    npc=N_NODES // NCORES,      # 2500 nodes per core
    npad=2560,                  # padded to 20 windows of 128
    ntbl=20096,                 # global node table rows (padded)
    gch=16,                     # tiles per gather chunk (2048 edges)
    grp=4,                      # tiles per m1-psum group (512 edges)
)


# ----------------------------------------------------------------- host prep
def _host_prep(h, edge_index, edge_attr, edge_mask, cfg):
    """Bucket+sort edges by (core, window, row); build per-core device arrays."""
    npc, npad = cfg['npc'], cfg['npad']
    nw = npad // 128
    E = edge_index.shape[1]

    rowg = edge_index[0].astype(np.int64)
    colg = edge_index[1].astype(np.int64)
    core = rowg // npc
    rl = rowg - core * npc            # row local to core, [0, npc)
    win = rl // 128                   # window within core

    cnt = np.zeros((NCORES, nw), np.int64)
    np.add.at(cnt, (core, win), 1)
    tw = np.maximum(1, -(-cnt.max(axis=0) // 128))      # tiles per window
    off = np.concatenate([[0], np.cumsum(tw)])          # tile offset per window
    T = int(off[-1])
    TE = T * 128

    # within each (core, window) bucket, order edges by COLUMN: the z1b
    # column-gather (42MB, random otherwise) becomes near-sequential in HBM,
    # while the row gather stays local regardless (a window's rows span only
    # a 64KB region of z1a). Bucketing by (core, win) is what correctness
    # needs; the within-bucket order is free to choose.
    order = np.lexsort((colg, win, core))
    sc, sw = core[order], win[order]
    bucket = sc * nw + sw
    # position of each sorted edge within its (core, window) bucket
    _, bstart, bcnt = np.unique(bucket, return_index=True, return_counts=True)
    pos = np.arange(E) - np.repeat(bstart, bcnt)
    slot = off[sw] * 128 + pos        # slot within the core's edge stream

    # tile -> window map
    tile_win = np.zeros(T, np.int64)
    for w in range(nw):
        tile_win[off[w]:off[w + 1]] = w

    ea_T = edge_attr.astype(np.float32).T               # [16, E]
    mask_v = edge_mask.astype(np.float32).reshape(-1)

    per_core = []
    for c in range(NCORES):
        m = sc == c
        sl = slot[m]
        oi = order[m]
        # pad slots get window-local gather indices (not row 0) so their
        # descriptors stay on the same HBM pages as neighboring real edges;
        # mask=0 zeroes their contribution regardless
        pad_idx = np.repeat(tile_win * 128, 128).astype(np.int16)
        zrow = pad_idx.copy()
        zcol = pad_idx.copy()
        rloc = np.zeros(TE, np.float32)
        msk = np.zeros(TE, np.float32)
        eafm = np.zeros((17, TE), np.float32)
        zrow[sl] = rl[oi].astype(np.int16)
        zcol[sl] = colg[oi].astype(np.int16)
        rloc[sl] = (rl[oi] - sw[m] * 128).astype(np.float32)
        msk[sl] = mask_v[oi]
        eafm[:16, sl] = ea_T[:, oi]
        eafm[16, :] = 1.0

        def wrap16(x):                # element i -> [i%16, i//16], replicated x8
            return np.tile(np.ascontiguousarray(x.reshape(-1, 16).T), (8, 1))

        per_core.append(dict(
            zrow_idx=wrap16(zrow),
            zcol_idx=wrap16(zcol),
            rowloc_pm=np.ascontiguousarray(rloc.reshape(T, 128).T),
            mask_pm=np.ascontiguousarray(msk.reshape(T, 128).T),
            ea_fm=eafm,
        ))
    return per_core, T, tile_win


# ------------------------------------------------------------- device build
_DEBUG_AGG = False
_STAGE = 99  # debug: 1=P0 2=+gathers 3=+m1 4=+transpose 5=+m2 6=+att 7=+agg 99=full


def _build(T, tile_win, cfg, act_fn, repeat=1):
    npad, ntbl = cfg['npad'], cfg['ntbl']
    nw = npad // 128
    gch, grp = cfg['gch'], cfg['grp']
    TE = T * 128
    nt_tbl = ntbl // 128

    nc = bacc.Bacc("TRN2", debug=False)

    # ---- inputs
    hT = nc.dram_tensor("hT", [D, ntbl], F32, kind="ExternalInput")
    hTmy_d = nc.dram_tensor("hTmy", [D, npad], F32, kind="ExternalInput")
    h_nm = nc.dram_tensor("h_nm", [npad, D], F32, kind="ExternalInput")
    flags_pm = nc.dram_tensor("flags_pm", [128, nw], F32, kind="ExternalInput")
    zrow_idx = nc.dram_tensor("zrow_idx", [128, TE // 16], I16, kind="ExternalInput")
    zcol_idx = nc.dram_tensor("zcol_idx", [128, TE // 16], I16, kind="ExternalInput")
    rowloc_d = nc.dram_tensor("rowloc_pm", [128, T], F32, kind="ExternalInput")
    mask_d = nc.dram_tensor("mask_pm", [128, T], F32, kind="ExternalInput")
    ea_d = nc.dram_tensor("ea_fm", [17, TE], F32, kind="ExternalInput")
    w1a_d = nc.dram_tensor("W1a", [D, D], F32, kind="ExternalInput")
    w1b_d = nc.dram_tensor("W1b", [D, D], F32, kind="ExternalInput")
    w1c_d = nc.dram_tensor("W1c_aug", [17, D], F32, kind="ExternalInput")
    w2_d = nc.dram_tensor("W2bf", [D, D], BF16, kind="ExternalInput")
    wab_d = nc.dram_tensor("WaB", [D, 4 * D], BF16, kind="ExternalInput")
    wn1h_d = nc.dram_tensor("Wn1h", [D, D], F32, kind="ExternalInput")
    wn1a_d = nc.dram_tensor("Wn1a", [D, D], F32, kind="ExternalInput")
    wn2_d = nc.dram_tensor("Wn2bf", [D, D], BF16, kind="ExternalInput")
    bn1_d = nc.dram_tensor("bn1", [D, 1], F32, kind="ExternalInput")
    bn2_d = nc.dram_tensor("bn2", [D, 1], F32, kind="ExternalInput")
    ba_d = nc.dram_tensor("ba", [D, 1], F32, kind="ExternalInput")
    if32_d = nc.dram_tensor("I_f32", [D, D], F32, kind="ExternalInput")
    ibf_d = nc.dram_tensor("I_bf16", [D, D], BF16, kind="ExternalInput")
    iota_d = nc.dram_tensor("iotaF", [D, D], F32, kind="ExternalInput")

    out_d = nc.dram_tensor("out_nm", [npad, D], F32, kind="ExternalOutput")
    dbg_agg = nc.dram_tensor("dbg_agg", [D, npad], F32, kind="ExternalOutput") if _DEBUG_AGG else None

    # ---- DRAM scratch node tables
    z1a_d = nc.dram_tensor("z1a_tbl", [npad, D], F32)
    z1b_d = nc.dram_tensor("z1b_tbl", [ntbl, D], F32)

    with tile.TileContext(nc) as tc:
        with (
            tc.tile_pool(name="consts", bufs=1) as cp,
            tc.tile_pool(name="zout", bufs=6) as zp,
            tc.tile_pool(name="streams", bufs=3) as sp,
            tc.tile_pool(name="small", bufs=8) as mp,
            tc.tile_pool(name="node", bufs=2) as npool,
            tc.tile_pool(name="pm1", bufs=2, space="PSUM") as pm1,
            tc.tile_pool(name="pmt", bufs=2, space="PSUM") as pmt,
            tc.tile_pool(name="pm2", bufs=2, space="PSUM") as pm2,
            tc.tile_pool(name="pagg", bufs=2, space="PSUM") as pagg,
        ):
            # ---- load constants
            def cload(dram, shape, dt):
                t = cp.tile(shape, dt, tag=dram.name)
                nc.sync.dma_start(out=t[:], in_=dram[:])
                return t

            w1a = cload(w1a_d, [D, D], F32)
            w1b = cload(w1b_d, [D, D], F32)
            w1c = cload(w1c_d, [17, D], F32)
            w2 = cload(w2_d, [D, D], BF16)
            wab = cload(wab_d, [D, 4 * D], BF16)
            wn1h = cload(wn1h_d, [D, D], F32)
            wn1a = cload(wn1a_d, [D, D], F32)
            wn2 = cload(wn2_d, [D, D], BF16)
            bn1 = cload(bn1_d, [D, 1], F32)
            bn2 = cload(bn2_d, [D, 1], F32)
            ba = cload(ba_d, [D, 1], F32)
            i32 = cload(if32_d, [D, D], F32)
            ibf = cload(ibf_d, [D, D], BF16)
            iota = cload(iota_d, [D, D], F32)
            rowloc = cload(rowloc_d, [128, T], F32)
            maskt = cload(mask_d, [128, T], F32)
            flagst = cload(flags_pm, [128, nw], F32)
            zrix = cload(zrow_idx, [128, TE // 16], I16)
            zcix = cload(zcol_idx, [128, TE // 16], I16)

            agg_sb = cp.tile([D, npad], F32, tag="agg_sb")
            hTmy = cp.tile([D, npad], F32, tag="hTmy")
            nc.sync.dma_start(out=hTmy[:], in_=hTmy_d[:])

            for _rep in range(repeat):
                # ---------------- P0: z1a then z1b tables, 4 node-tiles per
                # iteration (1 wide DMA in, 4 matmuls into one PSUM bank, 1 wide
                # copy, 1 3D-AP DMA out) — the naive per-128 version serialized
                # to 302us in the cost model, a third of the kernel.
                def table_batch(dst, lhs_tile, w_t, i, jn):
                    p = pm1.tile([128, jn * D], F32, tag="m1")
                    for j in range(jn):
                        nc.tensor.matmul(p[:, j * D:(j + 1) * D],
                                         lhsT=lhs_tile[:, j * D:(j + 1) * D],
                                         rhs=w_t[:], start=(j == 0),
                                         stop=(j == jn - 1))
                    zt = zp.tile([128, jn * D], F32, tag="zt")
                    nc.vector.tensor_copy(zt[:, :jn * D], p[:])
                    nc.sync.dma_start(
                        out=dst[i * 512:i * 512 + jn * D, :].rearrange(
                            "(j p) d -> p j d", p=128),
                        in_=zt[:, :jn * D].rearrange("p (j d) -> p j d", d=D))

                for i in range(npad // 512):
                    table_batch(z1a_d, hTmy[:, i * 512:(i + 1) * 512], w1a, i, 4)
                arem = npad // 512 * 512
                if arem < npad:
                    table_batch(z1a_d, hTmy[:, arem:npad], w1a, arem // 512,
                                (npad - arem) // D)
                for i in range(ntbl // 512):
                    hti = zp.tile([D, 512], F32, tag="hti")
                    nc.sync.dma_start(out=hti[:], in_=hT[:, i * 512:(i + 1) * 512])
                    table_batch(z1b_d, hti[:], w1b, i, 4)
                nrem = ntbl // 512 * 512
                if nrem < ntbl:
                    jn = (ntbl - nrem) // D
                    hti = zp.tile([D, 512], F32, tag="hti")
                    nc.sync.dma_start(out=hti[:, :jn * D], in_=hT[:, nrem:ntbl])
                    table_batch(z1b_d, hti[:], w1b, nrem // 512, jn)

                # ---------------- P1: edge loop
                agg_p = None
                for t0 in range(0, T, gch) if _STAGE >= 2 else []:
                    ntc = min(gch, T - t0)          # tiles this chunk
                    ne = ntc * 128
                    # gathers sub-chunked at 8 tiles (1024 indices): a single
                    # dma_gather above ~1024 descriptors overflows the SWDGE
                    # ring and wedges the device (HW-probed: 1024 ok, 4096 dies)
                    zrow_c = sp.tile([128, ntc, D], F32, tag="zrow_c")
                    zcol_c = sp.tile([128, ntc, D], F32, tag="zcol_c")
                    for s0 in range(0, ntc, 8):
                        sn = min(8, ntc - s0)
                        nc.gpsimd.dma_gather(
                            zrow_c[:, s0:s0 + sn, :], z1a_d[:],
                            zrix[:, (t0 + s0) * 8:(t0 + s0 + sn) * 8],
                            sn * 128, sn * 128, D)
                        nc.gpsimd.dma_gather(
                            zcol_c[:, s0:s0 + sn, :], z1b_d[:],
                            zcix[:, (t0 + s0) * 8:(t0 + s0 + sn) * 8],
                            sn * 128, sn * 128, D)
                    ea_c = sp.tile([17, ne], F32, tag="ea_c")
                    nc.sync.dma_start(out=ea_c[:], in_=ea_d[:, t0 * 128:t0 * 128 + ne])

                    if _STAGE == 2:
                        # keep gathers live: checksum into agg_sb
                        nc.vector.tensor_tensor(out=agg_sb[:, 0:128], in0=zrow_c[:, 0, :],
                                                in1=zcol_c[:, 0, :], op=mybir.AluOpType.add)
                        continue
                    m2a_list = []
                    attrC = mp.tile([128, gch], F32, tag="attrC")
                    for g0 in range(0, ntc, grp):
                        gn = min(grp, ntc - g0)
                        gw = gn * 128
                        m1b = pm1.tile([128, gw], F32, tag="m1")
                        for i in range(gn):
                            nc.tensor.matmul(
                                m1b[:, i * 128:(i + 1) * 128],
                                lhsT=ea_c[:, (g0 + i) * 128:(g0 + i + 1) * 128],
                                rhs=w1c[:], start=(i == 0), stop=(i == gn - 1))
                        # zrow+zcol summed on DVE (PE pushes serialized the m1
                        # bank behind the gathers; DVE adds let zea run ahead)
                        zs = sp.tile([128, gw], F32, tag="zs")
                        nc.vector.tensor_tensor(
                            out=zs[:],
                            in0=zrow_c[:, g0:g0 + gn, :].rearrange("p g d -> p (g d)"),
                            in1=zcol_c[:, g0:g0 + gn, :].rearrange("p g d -> p (g d)"),
                            op=mybir.AluOpType.add)
                        m1s = sp.tile([128, gw], F32, tag="m1s")
                        nc.vector.tensor_tensor(out=m1s[:], in0=zs[:], in1=m1b[:],
                                                op=mybir.AluOpType.add)
                        m1e = mp.tile([128, gw], BF16, tag="m1e")
                        nc.scalar.activation(m1e[:], m1s[:], act_fn)
                        if _STAGE == 3:
                            nc.vector.tensor_copy(agg_sb[:, 0:128], m1e[:, 0:128])
                            continue
                        m1tp = pmt.tile([128, gw], BF16, tag="mt")
                        for i in range(gn):
                            nc.tensor.matmul(m1tp[:, i * 128:(i + 1) * 128],
                                             lhsT=m1e[:, i * 128:(i + 1) * 128],
                                             rhs=ibf[:], is_transpose=True,
                                             start=(i == 0), stop=(i == gn - 1))
                        m1f = mp.tile([128, gw], BF16, tag="m1f")
                        nc.vector.tensor_copy(m1f[:], m1tp[:])
                        if _STAGE == 4:
                            nc.vector.tensor_copy(agg_sb[:, 0:128], m1f[:, 0:128])
                            continue
                        m2b = pm2.tile([128, gw], F32, tag="m2")
                        for i in range(gn):
                            sl = slice(i * 128, (i + 1) * 128)
                            nc.tensor.matmul(m2b[:, sl], lhsT=m1f[:, sl], rhs=w2[:],
                                             start=(i == 0), stop=(i == gn - 1))
                        m2a = mp.tile([128, gw], BF16, tag="m2a")
                        nc.scalar.activation(m2a[:], m2b[:], act_fn)
                        if _STAGE == 5:
                            nc.vector.tensor_copy(agg_sb[:, 0:128], m2a[:, 0:128])
                            continue
                        scr = mp.tile([128, gw], BF16, tag="scr")
                        nc.vector.tensor_tensor(out=scr[:], in0=m2a[:],
                                                in1=wab[:, :gw],
                                                op=mybir.AluOpType.mult)
                        nc.vector.tensor_reduce(
                            out=attrC[:, g0:g0 + gn],
                            in_=scr[:].rearrange("p (g d) -> p g d", d=D),
                            axis=mybir.AxisListType.X,
                            op=mybir.AluOpType.add)
                        m2a_list.append((g0, gn, m2a))
                    if _STAGE == 5:
                        continue
                    # one sigmoid per CHUNK: ACT table switches (silu<->sigmoid)
                    # were ~2 per group; hoisting to chunk level cuts them 4x
                    attC = mp.tile([128, gch], F32, tag="attC")
                    nc.scalar.activation(attC[:, :ntc], attrC[:, :ntc],
                                         mybir.ActivationFunctionType.Sigmoid,
                                         bias=ba[:, 0:1])
                    attmC = mp.tile([128, gch], F32, tag="attmC")
                    nc.vector.tensor_tensor(out=attmC[:, :ntc], in0=attC[:, :ntc],
                                            in1=maskt[:, t0:t0 + ntc],
                                            op=mybir.AluOpType.mult)
                    if _STAGE in (6, 60, 61):
                        nc.vector.tensor_copy(agg_sb[:, 0:1], attmC[:, 0:1])
                        continue
                    for g0, gn, m2a in m2a_list:
                        for i in range(gn):
                            t = t0 + g0 + i
                            sl = slice(i * 128, (i + 1) * 128)
                            # one-hot scaled by att*mask: folds the msg scale
                            # into the scatter operand (saves a DVE pass)
                            oem = mp.tile([128, D], BF16, tag="oem")
                            nc.vector.tensor_scalar(out=oem[:], in0=iota[:],
                                                    scalar1=rowloc[:, t:t + 1],
                                                    scalar2=attmC[:, g0 + i:g0 + i + 1],
                                                    op0=mybir.AluOpType.is_equal,
                                                    op1=mybir.AluOpType.mult)
                            w_ = int(tile_win[t])
                            first = (t == 0) or (tile_win[t - 1] != w_)
                            last = (t == T - 1) or (tile_win[t + 1] != w_)
                            if first:
                                agg_p = pagg.tile([128, D], F32, tag="agg")
                            nc.tensor.matmul(agg_p[:], lhsT=m2a[:, sl], rhs=oem[:],
                                             start=first, stop=last)
                            if last:
                                nc.vector.tensor_copy(
                                    agg_sb[:, w_ * 128:(w_ + 1) * 128], agg_p[:])

                if dbg_agg is not None:
                    nc.sync.dma_start(out=dbg_agg[:], in_=agg_sb[:])

                # ---------------- P2: node MLP (feature-major)
                if _STAGE < 8:
                    for q0 in range(0, npad, 128):
                        t = npool.tile([128, D], F32, tag="nh")
                        nc.sync.dma_start(out=t[:], in_=h_nm[q0:q0 + 128, :])
                        nc.sync.dma_start(out=out_d[q0:q0 + 128, :], in_=t[:])
                for q0 in range(0, npad, 512) if _STAGE >= 8 else []:
                    qn = min(512, npad - q0)
                    pu = pm1.tile([128, qn], F32, tag="m1")
                    nc.tensor.matmul(pu[:], lhsT=wn1h[:], rhs=hTmy[:, q0:q0 + qn],
                                     start=True, stop=False)
                    nc.tensor.matmul(pu[:], lhsT=wn1a[:], rhs=agg_sb[:, q0:q0 + qn],
                                     start=False, stop=True)
                    u1 = npool.tile([128, qn], BF16, tag="u1")
                    nc.scalar.activation(u1[:], pu[:], act_fn, bias=bn1[:, 0:1])
                    pup = pm2.tile([128, qn], F32, tag="m2")
                    nc.tensor.matmul(pup[:], lhsT=wn2[:], rhs=u1[:],
                                     start=True, stop=True)
                    updf = npool.tile([128, qn], F32, tag="updf")
                    nc.scalar.activation(updf[:], pup[:],
                                         mybir.ActivationFunctionType.Identity,
                                         bias=bn2[:, 0:1])
                    for i in range(qn // 128):
                        r0 = q0 + i * 128
                        pt = pagg.tile([128, D], F32, tag="agg")
                        nc.tensor.transpose(pt[:], updf[:, i * 128:(i + 1) * 128],
                                            i32[:])
                        nh = npool.tile([128, D], F32, tag="nh")
                        nc.sync.dma_start(out=nh[:], in_=h_nm[r0:r0 + 128, :])
                        so = npool.tile([128, D], F32, tag="so")
                        nc.vector.tensor_tensor(out=so[:], in0=pt[:], in1=nh[:],
                                                op=mybir.AluOpType.add)
                        oo = npool.tile([128, D], F32, tag="oo")
                        nc.vector.tensor_scalar(
                            out=oo[:], in0=so[:],
                            scalar1=flagst[:, r0 // 128:r0 // 128 + 1],
                            scalar2=None, op0=mybir.AluOpType.mult)
                        nc.sync.dma_start(out=out_d[r0:r0 + 128, :], in_=oo[:])

    nc.compile()
    return nc


# --------------------------------------------------------------- entry point
_CACHE = {}
_DBG = []


def kernel(h, edge_index, edge_attr, flags, edge_mask,
           W_m1, b_m1, W_m2, b_m2, W_a, b_a, W_n1, b_n1, W_n2, b_n2,
           cfg=None, act_fn=None, repeat=1, _return_raw=False, _sim=False):
    """Full inputs in, full output out. Shards edges over 8 NeuronCores."""

    cfg = dict(DEFAULT_CFG, **(cfg or {}))
    if act_fn is None:
        act_fn = mybir.ActivationFunctionType.Silu
    npc, npad, ntbl = cfg['npc'], cfg['npad'], cfg['ntbl']
    nw = npad // 128
    n = h.shape[0]

    h = np.asarray(h, np.float32)
    edge_index = np.asarray(edge_index, np.int32)
    edge_attr = np.asarray(edge_attr, np.float32)
    flags = np.asarray(flags, np.float32)
    edge_mask = np.asarray(edge_mask, np.float32)

    # fold b_m2 into... NOTE: reference has W_m2/b_m2 then W_a/b_a.
    per_core, T, tile_win = _host_prep(h, edge_index, edge_attr, edge_mask, cfg)

    key = (T, tuple(tile_win.tolist()), repeat, int(act_fn), n)
    if key not in _CACHE:
        _CACHE[key] = _build(T, tile_win, cfg, act_fn, repeat=repeat)
    nc = _CACHE[key]

    bf = ml_dtypes.bfloat16
    hTg = np.zeros((D, ntbl), np.float32)
    hTg[:, :n] = h.T

    shared = dict(
        hT=hTg,
        W1a=np.ascontiguousarray(W_m1[0:D].astype(np.float32)),
        W1b=np.ascontiguousarray(W_m1[D:2 * D].astype(np.float32)),
        W1c_aug=np.ascontiguousarray(
            np.vstack([W_m1[2 * D:2 * D + ED], b_m1[None, :]]).astype(np.float32)),
        W2bf=np.asarray(W_m2, np.float32).astype(bf),
        WaB=np.tile(np.asarray(W_a, np.float32).reshape(1, D), (D, 4)).astype(bf),
        Wn1h=np.ascontiguousarray(np.asarray(W_n1)[0:D].astype(np.float32)),
        Wn1a=np.ascontiguousarray(np.asarray(W_n1)[D:2 * D].astype(np.float32)),
        Wn2bf=np.asarray(W_n2, np.float32).astype(bf),
        bn1=np.asarray(b_n1, np.float32).reshape(D, 1),
        bn2=np.asarray(b_n2, np.float32).reshape(D, 1),
        ba=np.full((D, 1), float(np.asarray(b_a).reshape(-1)[0]), np.float32),
        I_f32=np.eye(D, dtype=np.float32),
        I_bf16=np.eye(D, dtype=bf),
        iotaF=np.tile(np.arange(D, dtype=np.float32), (D, 1)),
    )
    # b_m2 is all-zero in this problem's setup_inputs; the kernel does not
    # add it, so fail loudly if that ever changes.
    b_m2 = np.asarray(b_m2, np.float32)
    assert np.abs(b_m2).max() == 0.0, "b_m2 != 0 not supported by this kernel"

    in_maps = []
    for c in range(NCORES):
        base = c * npc
        hTmy = np.zeros((D, npad), np.float32)
        lim = min(npad, n - base)
        hTmy[:, :lim] = h.T[:, base:base + lim]
        h_nm = np.zeros((npad, D), np.float32)
        h_nm[:lim] = h[base:base + lim]
        fl = np.zeros(npad, np.float32)
        fl[:min(npc, n - base)] = flags.reshape(-1)[base:base + min(npc, n - base)]
        flags_pm = np.ascontiguousarray(fl.reshape(nw, 128).T)
        pc = per_core[c]
        in_maps.append(dict(
            shared,
            hTmy=hTmy, h_nm=h_nm, flags_pm=flags_pm,
            zrow_idx=pc['zrow_idx'], zcol_idx=pc['zcol_idx'],
            rowloc_pm=pc['rowloc_pm'], mask_pm=pc['mask_pm'], ea_fm=pc['ea_fm'],
        ))

    if _sim:
        from concourse.bass_interp import CoreSim
        core_outs = []
        _DBG.clear()
        for c in range(NCORES):
            sim = CoreSim(nc)
            for k, v in in_maps[c].items():
                sim.tensor(k)[:] = v
            sim.simulate()
            core_outs.append(np.array(sim.tensor("out_nm")))
            if _DEBUG_AGG:
                _DBG.append(np.array(sim.tensor("dbg_agg")))
    else:
        from concourse.bass_utils import run_bass_kernel_spmd
        res = run_bass_kernel_spmd(nc, in_maps, core_ids=list(range(NCORES)))
        if _return_raw:
            return res
        core_outs = [res.results[c]["out_nm"] for c in range(NCORES)]

    out = np.zeros((n, D), np.float32)
    for c in range(NCORES):
        base = c * npc
        lim = min(npc, n - base)
        out[base:base + lim] = core_outs[c][:lim]
    return out



# revision 54
# speedup vs baseline: 10713.3809x; 10713.3809x over previous
"""GNN message-passing layer (EGNN-style GCL) on 8 Trainium2 NeuronCores.

Strategy: shard edges across the 8 cores BY DESTINATION ROW (row =
edge_index[0], the segment-sum target); each core owns a 2500-node output
partition and its incoming edges, so no collectives are needed.

v2 pipeline (per core), all matmuls bf16, m1 kept feature-major:

  P0: comb_w[w] = [W1c_aug (17 rows); z1a_win (111 rows)]  per window w
        where z1a_win = h_win @ W_m1[0:128]    (built on PE, kept in SBUF)
      z1b_sb = h_all @ W_m1[128:256]           (bf16, SBUF-resident,
                                                node-wrapped for SBUF gather)
  P1 (edge loop, edges sorted by row into 111-node windows, tiles of 128):
      zcol_fm = dma_gather(z1b_sb, col, transpose=True)   # [128f, e] bf16
      m1_fm   = comb_w[w]^T @ [ea_aug; onehot_row]        # one matmul: W_m1
                + I^T @ zcol_fm                           # edge-attr + zrow
      m1e     = silu(m1_fm)                               # ACT, psum->sbuf
      m2      = m1e^T(lhsT) @ W_m2 -> edge-major psum; m2a = silu(m2)
      attr[e] = ttr: sum_f(m2a*Wa) + (b_a - 30*(1-mask))  # DVE, bf16 2x
      att     = 0.5*tanh(0.5*attr) + 0.5                  # ==sigmoid; tanh
                                                          # shares silu's ACT
                                                          # table (no reloads)
      oem     = (iota==rowloc) * att                      # scatter one-hot
      agg_fm[:, win] += m2a^T @ oem                       # PE matmul scatter
  P2 (node MLP, feature-major): as v1 but bf16 weights.

Pad edges carry rowloc=120 (>=111) so they scatter into discarded one-hot
columns; no mask multiply needed for padding.
"""
import sys

for _p in ('/opt/trn_rl_repo', '/root/.axon_site/_ro/trn_rl_repo'):
    if _p not in sys.path:
        sys.path.append(_p)

import numpy as np
import ml_dtypes

from concourse import bacc
import concourse.mybir as mybir
import concourse.tile as tile

F32 = mybir.dt.float32
BF16 = mybir.dt.bfloat16
I16 = mybir.dt.int16
F8 = mybir.dt.bfloat16  # fp8 disabled: HW bring-up

# problem constants (hardcoded per spec)
N_NODES, N_EDGES = 20000, 640000
D, ED = 128, 16
NCORES = 8

DEFAULT_CFG = dict(
    n_nodes=N_NODES,
    npc=N_NODES // NCORES,      # 2500 nodes per core
    win=111,                    # nodes per scatter window (17+111=128 rows
                                # in the combined m1 lhsT)
    nw=23,                      # windows per core (23*111 = 2553 >= 2500)
    npad=2560,                  # P2 node padding (20 tiles of 128)
    ntbl=20096,                 # global node table rows (157*128)
    gch=24,                     # tiles per edge chunk
    grp=4,                      # tiles per m1/m2 psum group
)


# ----------------------------------------------------------------- host prep
def _host_prep(h, edge_index, edge_attr, edge_mask, cfg):
    """Bucket+sort edges by (core, window, col); build per-core arrays."""
    npc, win, nw = cfg['npc'], cfg['win'], cfg['nw']
    E = edge_index.shape[1]

    rowg = edge_index[0].astype(np.int64)
    colg = edge_index[1].astype(np.int64)
    core = rowg // npc
    rl = rowg - core * npc            # row local to core, [0, npc)
    w = rl // win                     # window within core
    slot = rl - w * win               # slot within window, [0, win)

    cnt = np.zeros((NCORES, nw), np.int64)
    np.add.at(cnt, (core, w), 1)
    tw = np.maximum(1, -(-cnt.max(axis=0) // 128))      # tiles per window
    off = np.concatenate([[0], np.cumsum(tw)])          # tile offset per window
    T = int(off[-1])
    TE = T * 128

    # within each (core, window) bucket order edges by column so the z1b
    # gather walks the table near-sequentially
    order = np.lexsort((colg, w, core))
    sc, sw = core[order], w[order]
    bucket = sc * nw + sw
    _, bstart, bcnt = np.unique(bucket, return_index=True, return_counts=True)
    pos = np.arange(E) - np.repeat(bstart, bcnt)
    eslot = off[sw] * 128 + pos       # slot within the core's edge stream

    tile_win = np.zeros(T, np.int64)
    for ww in range(nw):
        tile_win[off[ww]:off[ww + 1]] = ww

    ea_T = edge_attr.astype(np.float32).T               # [16, E]
    mask_v = edge_mask.astype(np.float32).reshape(-1)
    bf = ml_dtypes.bfloat16

    def wrap16(x):                    # element i -> [i%16, i//16], replicated x8
        return np.tile(np.ascontiguousarray(x.reshape(-1, 16).T), (8, 1))

    per_core = []
    for c in range(NCORES):
        m = sc == c
        sl = eslot[m]
        oi = order[m]
        zcol = np.zeros(TE, np.int16)
        zcol[sl] = colg[oi].astype(np.int16)
        # rhs_pack rows: 0:16 edge_attr^T, 16 bias ones, 17:128 row one-hot
        rhs = np.zeros((128, TE), np.float32)
        rhs[16, :] = 1.0
        rhs[:16, sl] = ea_T[:, oi]
        rhs[17 + slot[oi], sl] = 1.0
        rloc = np.full(TE, 120.0, np.float32)           # pads -> discard bin
        rloc[sl] = slot[oi].astype(np.float32)
        mk = np.zeros(TE, np.float32)                   # pad mask = 0
        mk[sl] = mask_v[oi]
        per_core.append(dict(
            zcol_idx=wrap16(zcol),
            rhs_pack=rhs.astype(bf),
            rowloc_pm=np.ascontiguousarray(rloc.reshape(T, 128).T),
            mask_pm=np.ascontiguousarray(mk.reshape(T, 128).T),
        ))
    return per_core, T, tile_win


# ------------------------------------------------------------- device build
def _build(T, tile_win, cfg, act_fn):
    npc, win, nw = cfg['npc'], cfg['win'], cfg['nw']
    npad, ntbl = cfg['npad'], cfg['ntbl']
    gch, grp = cfg['gch'], cfg['grp']
    TE = T * 128
    nwp = npad // 128                 # P2 flag windows (128-aligned)
    nrank = ntbl // 128               # z1b table ranks

    nc = bacc.Bacc("TRN2", debug=False)

    # ---- inputs (packed to minimize HWDGE descriptor-gen serialization:
    # every dma_start pays ~650ns on the HWDGE regardless of size)
    # wpack_bf blocks: w1a, w2, wab, wn1h, wn2, ibf, iota
    wbf_d = nc.dram_tensor("wpack_bf", [128, 7 * D], BF16, kind="ExternalInput")
    # wpack_f32 blocks: wn1a(128), i32(128), bn1(1), bn2(1), flags(nwp)
    wf32_d = nc.dram_tensor("wpack_f32", [128, 2 * D + 2 + nwp], F32,
                            kind="ExternalInput")
    # f8pack blocks: comb_w0 (nw*D), w1b (D), hT_all (ntbl)
    f8_d = nc.dram_tensor("f8pack", [128, nw * D + D + ntbl], F8,
                          kind="ExternalInput")
    # hbf_pack blocks: hT_myw (nw*D), hT_my (npad)
    hbf_d = nc.dram_tensor("hbf_pack", [128, nw * D + npad], BF16,
                           kind="ExternalInput")
    # rm_pack blocks: rowloc (T), maskbias (T)
    rm_d = nc.dram_tensor("rm_pack", [128, 2 * T], F32, kind="ExternalInput")
    zcol_idx = nc.dram_tensor("zcol_idx", [128, TE // 16], I16, kind="ExternalInput")
    rhs_d = nc.dram_tensor("rhs_pack", [128, TE], F8, kind="ExternalInput")
    h_nm = nc.dram_tensor("h_nm", [npad, D], F32, kind="ExternalInput")

    out_d = nc.dram_tensor("out_nm", [npad, D], F32, kind="ExternalOutput")
    z1b_d = nc.dram_tensor("z1b_tbl", [ntbl, D], BF16)

    with tile.TileContext(nc) as tc:
        with (
            tc.tile_pool(name="consts", bufs=1) as cp,
            tc.tile_pool(name="streams", bufs=3) as sp,
            tc.tile_pool(name="small", bufs=13) as mp,
            tc.tile_pool(name="oemp", bufs=30) as op,
            tc.tile_pool(name="scrp", bufs=3) as scp,
            tc.tile_pool(name="node", bufs=2) as npool,
            tc.tile_pool(name="pm1", bufs=2, space="PSUM") as pm1,
            tc.tile_pool(name="pm2", bufs=2, space="PSUM") as pm2,
            tc.tile_pool(name="pmt", bufs=2, space="PSUM") as pmt,
            tc.tile_pool(name="pagg", bufs=2, space="PSUM") as pagg,
        ):
            # ---- load constants (few large packed DMAs)
            def cload(dram, shape, dt):
                t = cp.tile(shape, dt, tag=dram.name)
                nc.sync.dma_start(out=t[:], in_=dram[:])
                return t

            wbf = cload(wbf_d, [128, 7 * D], BF16)
            w1a, w2, wab, wn1h, wn2, ibf, iota = (
                wbf[:, i * D:(i + 1) * D] for i in range(7))
            wab4 = cp.tile([128, 4 * D], BF16, tag="wab4")
            for _i in range(4):
                nc.vector.tensor_copy(wab4[:, _i * D:(_i + 1) * D], wab)
            wf32 = cload(wf32_d, [128, 2 * D + 2 + nwp], F32)
            wn1a = wf32[:, 0:D]
            i32 = wf32[:, D:2 * D]
            bn1 = wf32[:, 2 * D:2 * D + 1]
            bn2 = wf32[:, 2 * D + 1:2 * D + 2]
            flagst = wf32[:, 2 * D + 2:2 * D + 2 + nwp]
            f8p = cload(f8_d, [128, nw * D + D + ntbl], F8)
            combw = f8p[:, 0:nw * D]
            w1b = f8p[:, nw * D:nw * D + D]
            hTa = f8p[:, nw * D + D:]
            hbf = cload(hbf_d, [128, nw * D + npad], BF16)
            hTmyw = hbf[:, 0:nw * D]
            hTmy = hbf[:, nw * D:]
            rm = cload(rm_d, [128, 2 * T], F32)
            rowloc = rm[:, 0:T]
            mbias = rm[:, T:]
            zcix = cload(zcol_idx, [128, TE // 16], I16)

            agg_sb = cp.tile([D, npad], F32, tag="agg_sb")
            z1b_sb = cp.tile([128, nrank * D], BF16, tag="z1b_sb")
            # zero the agg tail beyond nw*win so P2 reads defined values
            if nw * win < npad:
                nc.vector.memset(agg_sb[:, nw * win:npad], 0.0)

            # ---------------- P0a: z1a windows into comb_w rows 17:128
            for w in range(nw):
                p = pagg.tile([128, 4 * D], F32, tag="agg")
                nc.tensor.matmul(p[:, 0:D], lhsT=hTmyw[:, w * D:(w + 1) * D],
                                 rhs=w1a[:], start=True, stop=True)
                # psum rows 0:16 are exactly zero (hT_myw has 17 zero lead
                # cols), so adding the full tile leaves the W1c rows intact
                nc.vector.tensor_tensor(out=combw[:, w * D:(w + 1) * D],
                                        in0=combw[:, w * D:(w + 1) * D],
                                        in1=p[:, 0:D],
                                        op=mybir.AluOpType.add)

            # ---------------- P0b: z1b table (node-wrapped bf16, SBUF)
            for r0 in range(0, nrank, 4):
                rn = min(4, nrank - r0)
                p = pm1.tile([128, 4 * D], F32, tag="m1")
                for j in range(rn):
                    nc.tensor.matmul(p[:, j * D:(j + 1) * D],
                                     lhsT=hTa[:, (r0 + j) * D:(r0 + j + 1) * D],
                                     rhs=w1b[:], start=(j == 0),
                                     stop=(j == rn - 1))
                # split the psum->sbuf converts across DVE and ACT
                # (GPSIMD cannot access PSUM on real hardware)
                dst = z1b_sb[:, r0 * D:(r0 + rn) * D]
                if (r0 // 4) % 2 == 0:
                    nc.vector.tensor_copy(dst, p[:, :rn * D])
                else:
                    nc.scalar.activation(dst, p[:, :rn * D],
                                         mybir.ActivationFunctionType.Copy)
                nc.sync.dma_start(
                    out=z1b_d[r0 * 128:(r0 + rn) * 128, :].rearrange(
                        "(r p) d -> p r d", p=128),
                    in_=z1b_sb[:, r0 * D:(r0 + rn) * D].rearrange(
                        "p (r d) -> p r d", d=D))

            # ---------------- P1: edge loop. The att chain (ttr -> tanh ->
            # attC -> oem -> scatter) for chunk i is emitted during chunk
            # i+1 (software pipeline): it otherwise head-of-line blocks the
            # ACT/PE/DVE queues at every chunk tail. oem generation is split
            # DVE/Pool to balance engine load.
            agg_p = None
            pend = []              # [(t0, ntc, m2a_list, attr)] two-deep queue

            def emit_att_oem(t0, ntc, m2a_list, attr):
                # att = sigmoid(attr) = 0.5*tanh(0.5*attr) + 0.5; tanh shares
                # silu's activation table so no table reloads occur
                nc.vector.tensor_tensor(out=attr[:, :ntc],
                                        in0=attr[:, :ntc],
                                        in1=mbias[:, t0:t0 + ntc],
                                        op=mybir.AluOpType.add)
                th = mp.tile([128, gch], F32, tag="th")
                nc.scalar.activation(th[:, :ntc], attr[:, :ntc],
                                     mybir.ActivationFunctionType.Tanh,
                                     scale=0.5)
                attC = mp.tile([128, gch], F32, tag="attC")
                nc.vector.tensor_scalar(out=attC[:, :ntc], in0=th[:, :ntc],
                                        scalar1=0.5, scalar2=0.5,
                                        op0=mybir.AluOpType.mult,
                                        op1=mybir.AluOpType.add)
                oems = []
                for g0, gn, m2a in m2a_list:
                    for i in range(gn):
                        t = t0 + g0 + i
                        oem = op.tile([128, D], BF16, tag="oem")
                        eng = nc.vector
                        eng.tensor_scalar(
                            out=oem[:], in0=iota[:],
                            scalar1=rowloc[:, t:t + 1],
                            scalar2=attC[:, g0 + i:g0 + i + 1],
                            op0=mybir.AluOpType.is_equal,
                            op1=mybir.AluOpType.mult)
                        oems.append((t, m2a, slice(i * 128, (i + 1) * 128),
                                     oem))
                return oems

            done_wins = []         # (win, quad_tile, col0, emit_chunk)
            quad = {}              # current psum quad tile (4 windows/bank)

            def emit_scatter(oems):
                # rank the scatter matmuls behind the MLP stream in the
                # scheduler's priority heap: they depend on the slow att
                # chain, and scheduled early they head-of-line block the PE.
                # Completed windows stay parked in their PSUM tile; the
                # psum->sbuf copy is deferred until just before the P2 group
                # that reads them (by then the scatters are long done, so
                # the copy never blocks the DVE queue).
                p0 = tc.cur_priority
                tc.cur_priority = p0 + 120
                for t, m2a, sl, oem in oems:
                    w_ = int(tile_win[t])
                    first = (t == 0) or (tile_win[t - 1] != w_)
                    last = (t == T - 1) or (tile_win[t + 1] != w_)
                    if first and w_ % 4 == 0:
                        aggq = pagg.tile([128, 4 * D], F32, tag="agg")
                        quad['t'] = aggq
                    c0 = (w_ % 4) * D
                    nc.tensor.matmul(quad['t'][:, c0:c0 + D],
                                     lhsT=m2a[:, sl], rhs=oem[:],
                                     start=first, stop=last)
                    if last and (w_ % 4 == 3 or w_ == nw - 1):
                        # whole quad complete: record one strided flush
                        done_wins.append((w_ - w_ % 4, w_ % 4 + 1,
                                          quad['t'], t // gch))
                tc.cur_priority = p0

            def flush_agg(upto_age=None):
                for w0, nq, tile_, ec in list(done_wins):
                    if upto_age is not None and ec > upto_age:
                        continue
                    nc.vector.tensor_copy(
                        agg_sb[:, w0 * win:(w0 + nq) * win].rearrange(
                            "p (q c) -> p q c", c=win),
                        tile_[:, :nq * 128].rearrange(
                            "p (q c) -> p q c", c=128)[:, :, 0:win])
                    done_wins.remove((w0, nq, tile_, ec))

            # ---------------- P2: node MLP (feature-major), emitted per
            # 512-node group as soon as the agg windows it reads are fully
            # scattered (hides the node MLP under the edge loop)
            def emit_p2(q0):
                # demoted rank: P2 has slack; at normal rank its out-DMA
                # blocks the SP queue ahead of the rhs_c prefetches
                p0 = tc.cur_priority
                tc.cur_priority = p0 + 300
                flush_agg()
                qn = min(512, npad - q0)
                nb = qn // 128
                pu = pm1.tile([128, 512], F32, tag="m1")
                nc.tensor.matmul(pu[:, :qn], lhsT=wn1h,
                                 rhs=hTmy[:, q0:q0 + qn],
                                 start=True, stop=False)
                nc.tensor.matmul(pu[:, :qn], lhsT=wn1a,
                                 rhs=agg_sb[:, q0:q0 + qn],
                                 start=False, stop=True)
                u1 = npool.tile([128, 512], BF16, tag="u1")
                nc.scalar.activation(u1[:, :qn], pu[:, :qn], act_fn, bias=bn1)
                pup = pm2.tile([128, 512], F32, tag="m2")
                nc.tensor.matmul(pup[:, :qn], lhsT=wn2, rhs=u1[:, :qn],
                                 start=True, stop=True)
                updf = npool.tile([128, 512], F32, tag="updf")
                nc.scalar.activation(updf[:, :qn], pup[:, :qn],
                                     mybir.ActivationFunctionType.Identity,
                                     bias=bn2)
                ptr = pm2.tile([128, 512], F32, tag="m2")
                for i in range(nb):
                    nc.tensor.transpose(ptr[:, i * 128:(i + 1) * 128],
                                        updf[:, i * 128:(i + 1) * 128], i32)
                nh4 = npool.tile([128, 512], F32, tag="nh")
                nc.sync.dma_start(
                    out=nh4[:, :qn].rearrange("p (a d) -> p a d", d=128),
                    in_=h_nm[q0:q0 + qn, :].rearrange("(a p) d -> p a d",
                                                      p=128))
                so = npool.tile([128, 512], F32, tag="so")
                nc.vector.tensor_tensor(out=so[:, :qn], in0=ptr[:, :qn],
                                        in1=nh4[:, :qn],
                                        op=mybir.AluOpType.add)
                for i in range(nb):
                    blk = q0 // 128 + i
                    nc.vector.tensor_scalar(
                        out=so[:, i * 128:(i + 1) * 128],
                        in0=so[:, i * 128:(i + 1) * 128],
                        scalar1=flagst[:, blk:blk + 1],
                        scalar2=None, op0=mybir.AluOpType.mult)
                nc.sync.dma_start(
                    out=out_d[q0:q0 + qn, :].rearrange("(a p) d -> p a d",
                                                       p=128),
                    in_=so[:, :qn].rearrange("p (a d) -> p a d", d=128))
                tc.cur_priority = p0

            # P2 group q reads agg windows [q0//win, (q0+511)//win]; window w
            # is fully scattered two chunks after its last tile's chunk
            last_tile = {}
            for t in range(T):
                last_tile[int(tile_win[t])] = t
            p2_ready = {}
            p2_pending = []
            n_ci = (T + gch - 1) // gch
            for q0 in range(0, npad, 512):
                whi = min(nw - 1, ((q0 + 511) // win) | 3)
                p2_ready[q0] = last_tile[whi] // gch + 4
                if p2_ready[q0] >= n_ci:
                    p2_ready[q0] = -1          # after the drain
                p2_pending.append(q0)

            for t0 in range(0, T, gch):
                ntc = min(gch, T - t0)
                zcol_c = sp.tile([128, gch * 128], BF16, tag="zcol_c")
                for s0 in range(0, ntc, 8):
                    sn = min(8, ntc - s0)
                    nc.gpsimd.dma_gather(
                        zcol_c[:, s0 * 128:(s0 + sn) * 128].rearrange(
                            "p (o e) -> p o e", o=sn),
                        z1b_d[:],
                        zcix[:, (t0 + s0) * 8:(t0 + s0 + sn) * 8],
                        sn * 128, sn * 128, D)
                rhs_c = sp.tile([128, gch * 128], F8, tag="rhs_c")
                nc.sync.dma_start(out=rhs_c[:, :ntc * 128],
                                  in_=rhs_d[:, t0 * 128:(t0 + ntc) * 128])

                if len(pend) == 2:
                    oems = emit_att_oem(*pend.pop(0))
                else:
                    oems = None

                attr = mp.tile([128, gch], F32, tag="attr")
                m2a_list = []

                def emit_m1(g0):
                    gn = min(grp, ntc - g0)
                    m1p = pm1.tile([128, grp * 128], F32, tag="m1")
                    for i in range(gn):
                        w = int(tile_win[t0 + g0 + i])
                        sl = slice(i * 128, (i + 1) * 128)
                        nc.tensor.matmul(
                            m1p[:, sl],
                            lhsT=rhs_c[:, (g0 + i) * 128:(g0 + i + 1) * 128],
                            rhs=combw[:, w * D:(w + 1) * D],
                            start=(i == 0), stop=False)
                    for i in range(gn):
                        sl = slice(i * 128, (i + 1) * 128)
                        nc.tensor.matmul(
                            m1p[:, sl],
                            lhsT=ibf[:],
                            rhs=zcol_c[:, (g0 + i) * 128:(g0 + i + 1) * 128],
                            start=False, stop=(i == gn - 1))
                    m1e = sp.tile([128, grp * 128], BF16, tag="m1e")
                    nc.scalar.activation(m1e[:, :gn * 128], m1p[:, :gn * 128],
                                         act_fn)
                    m1tp = pmt.tile([128, grp * 128], BF16, tag="mt")
                    for i in range(gn):
                        sl = slice(i * 128, (i + 1) * 128)
                        nc.tensor.matmul(m1tp[:, sl], lhsT=m1e[:, sl],
                                         rhs=ibf[:], is_transpose=True,
                                         start=(i == 0), stop=(i == gn - 1))
                    m1f = sp.tile([128, grp * 128], BF16, tag="m1f")
                    nc.vector.tensor_copy(m1f[:, :gn * 128], m1tp[:, :gn * 128])
                    return m1f

                def emit_m2(g0, m1e):
                    gn = min(grp, ntc - g0)
                    gw = gn * 128
                    m2p = pm2.tile([128, grp * 128], F32, tag="m2")
                    for i in range(gn):
                        sl = slice(i * 128, (i + 1) * 128)
                        nc.tensor.matmul(m2p[:, sl], lhsT=m1e[:, sl], rhs=w2[:],
                                         start=(i == 0), stop=(i == gn - 1))
                    m2a = mp.tile([128, grp * 128], BF16, tag="m2a")
                    nc.scalar.activation(m2a[:, :gw], m2p[:, :gw], act_fn)
                    scrd = scp.tile([128, grp * 128], BF16, tag="scrd")
                    nc.vector.tensor_tensor(out=scrd[:, :gw], in0=m2a[:, :gw],
                                            in1=wab4[:, :gw],
                                            op=mybir.AluOpType.mult)
                    nc.vector.tensor_reduce(
                        out=attr[:, g0:g0 + gn],
                        in_=scrd[:, :gw].rearrange("p (g d) -> p g d", d=D),
                        axis=mybir.AxisListType.X,
                        op=mybir.AluOpType.add)
                    m2a_list.append((g0, gn, m2a))

                # one-group software pipeline: m2(g) is emitted after m1(g+1)
                # so the PE never waits on ACT's silu round-trip; the prior
                # chunk's scatter batch slots in after two m1 groups, by which
                # time its oem stream (DVE) has run ahead of the PE
                prev = None
                for gi, g0 in enumerate(range(0, ntc, grp)):
                    m1e = emit_m1(g0)
                    if gi == 1 and oems is not None:
                        emit_scatter(oems)
                        oems = None
                    if prev is not None:
                        emit_m2(*prev)
                    prev = (g0, m1e)
                if oems is not None:        # short tail chunk
                    emit_scatter(oems)
                    oems = None
                if prev is not None:
                    emit_m2(*prev)

                pend.append((t0, ntc, m2a_list, attr))

                # interleave P2 groups whose agg windows completed scattering
                ci = t0 // gch
                flush_agg(upto_age=ci - 6)
                for q0 in list(p2_pending):
                    if p2_ready[q0] == ci:
                        emit_p2(q0)
                        p2_pending.remove(q0)
            for p in pend:
                emit_scatter(emit_att_oem(*p))
            for q0 in p2_pending:
                emit_p2(q0)

    nc.compile()
    return nc


# --------------------------------------------------------------- entry point
_CACHE = {}


def kernel(h, edge_index, edge_attr, flags, edge_mask,
           W_m1, b_m1, W_m2, b_m2, W_a, b_a, W_n1, b_n1, W_n2, b_n2,
           cfg=None, act_fn=None, _sim=False, _sim_cores=None):
    """Full inputs in, full output out. Shards edges over 8 NeuronCores."""

    cfg = dict(DEFAULT_CFG, **(cfg or {}))
    if act_fn is None:
        act_fn = mybir.ActivationFunctionType.Silu
    npc, win, nw = cfg['npc'], cfg['win'], cfg['nw']
    npad, ntbl = cfg['npad'], cfg['ntbl']
    nwp = npad // 128
    n = h.shape[0]
    bf = ml_dtypes.bfloat16

    h = np.asarray(h, np.float32)
    edge_index = np.asarray(edge_index, np.int32)
    edge_attr = np.asarray(edge_attr, np.float32)
    flags = np.asarray(flags, np.float32)
    edge_mask = np.asarray(edge_mask, np.float32)

    per_core, T, tile_win = _host_prep(h, edge_index, edge_attr, edge_mask, cfg)

    key = (T, tuple(tile_win.tolist()), int(act_fn), n)
    if key not in _CACHE:
        _CACHE[key] = _build(T, tile_win, cfg, act_fn)
    nc = _CACHE[key]

    # b_m2 is all-zero in this problem's setup_inputs; the kernel does not
    # add it, so fail loudly if that ever changes.
    b_m2 = np.asarray(b_m2, np.float32)
    assert np.abs(b_m2).max() == 0.0, "b_m2 != 0 not supported by this kernel"
    b_a_f = float(np.asarray(b_a).reshape(-1)[0])

    hTg = np.zeros((D, ntbl), np.float32)
    hTg[:, :n] = h.T

    # comb_w0: W_m1 edge-attr block + bias row in rows 0:17, zeros below
    # (P0a fills rows 17:128 with the per-window z1a table)
    w1c_aug = np.vstack([np.asarray(W_m1)[2 * D:2 * D + ED],
                         np.asarray(b_m1)[None, :]]).astype(np.float32)
    combw0 = np.zeros((128, nw * D), np.float32)
    for w in range(nw):
        combw0[:17, w * D:(w + 1) * D] = w1c_aug

    f8 = bf
    # wpack_bf blocks: w1a, w2, wab, wn1h, wn2, ibf, iota
    wpack_bf = np.concatenate([
        np.ascontiguousarray(np.asarray(W_m1)[0:D]).astype(np.float32),
        np.asarray(W_m2, np.float32),
        np.tile(np.asarray(W_a, np.float32).reshape(1, D), (D, 1)),
        np.ascontiguousarray(np.asarray(W_n1)[0:D]).astype(np.float32),
        np.asarray(W_n2, np.float32),
        np.eye(D, dtype=np.float32),
        np.tile(np.arange(D, dtype=np.float32), (D, 1)),
    ], axis=1).astype(bf)
    # f8pack blocks: comb_w0, w1b, hT_all
    f8pack = np.concatenate([
        combw0,
        np.ascontiguousarray(np.asarray(W_m1)[D:2 * D]).astype(np.float32),
        hTg,
    ], axis=1).astype(f8)

    in_maps = []
    for c in range(NCORES):
        base = c * npc
        hmy = h[base:base + npc]                      # [2500, 128]
        hT_my = np.zeros((D, npad), np.float32)
        hT_my[:, :npc] = hmy.T
        h_nm = np.zeros((npad, D), np.float32)
        h_nm[:npc] = hmy
        # per-window h with 17 leading zero columns (z1a lands on rows 17:128)
        hT_myw = np.zeros((D, nw * D), np.float32)
        for w in range(nw):
            lo = w * win
            hi = min(lo + win, npc)
            hT_myw[:, w * D + 17:w * D + 17 + hi - lo] = hmy.T[:, lo:hi]
        fl = np.zeros(npad, np.float32)
        fl[:npc] = flags.reshape(-1)[base:base + npc]
        # wpack_f32 blocks: wn1a, i32, bn1, bn2, flags
        wpack_f32 = np.concatenate([
            np.ascontiguousarray(np.asarray(W_n1)[D:2 * D]).astype(np.float32),
            np.eye(D, dtype=np.float32),
            np.asarray(b_n1, np.float32).reshape(D, 1),
            np.asarray(b_n2, np.float32).reshape(D, 1),
            np.ascontiguousarray(fl.reshape(nwp, 128).T),
        ], axis=1)
        pc = per_core[c]
        maskbias = b_a_f - 30.0 * (1.0 - pc['mask_pm'])
        in_maps.append(dict(
            wpack_bf=wpack_bf,
            wpack_f32=wpack_f32,
            f8pack=f8pack,
            hbf_pack=np.concatenate([hT_myw, hT_my], axis=1).astype(bf),
            rm_pack=np.concatenate(
                [pc['rowloc_pm'], maskbias], axis=1).astype(np.float32),
            h_nm=h_nm,
            zcol_idx=pc['zcol_idx'], rhs_pack=pc['rhs_pack'],
        ))

    if _sim:
        from concourse.bass_interp import CoreSim
        core_outs = [None] * NCORES
        for c in (_sim_cores if _sim_cores is not None else range(NCORES)):
            sim = CoreSim(nc)
            for k, v in in_maps[c].items():
                sim.tensor(k)[:] = v
            sim.simulate()
            core_outs[c] = np.array(sim.tensor("out_nm"))
    else:
        from concourse.bass_utils import run_bass_kernel_spmd
        res = run_bass_kernel_spmd(nc, in_maps, core_ids=list(range(NCORES)))
        core_outs = [res.results[c]["out_nm"] for c in range(NCORES)]

    out = np.zeros((n, D), np.float32)
    for c in range(NCORES):
        base = c * npc
        lim = min(npc, n - base)
        if core_outs[c] is not None:
            out[base:base + lim] = core_outs[c][:lim]
    return out
